# revision 44
# baseline (speedup 1.0000x reference)
"""Trainium2 Bass kernel for the multi-agent attention critic.

Strategy: data-parallel over the batch dim across 8 NeuronCores
(4096 samples/core). Feature-major ([feature, batch] tiles, batch on
the free dim) in bf16 with fp32 PSUM accumulation.

v2 restructure vs the earlier baseline:
  - vals are computed LATE (in the weighted phase): the Wv matmul output
    stays in PSUM and a fused DVE scalar_tensor_tensor computes
    (valsp + bv) * wbc in one op; the relu commutes with the softmax
    weight (w >= 0) so a cheap all-bf16 4x-mode relu follows. This
    removes all seven vals ACT evacuations per tile.
  - softmax runs in transposed (batch-on-partitions) space: ee=exp(L)
    [32, BT] -> DMA-transpose -> [128, 4, 32]; segment sums become a
    strided DVE tensor_reduce (177 ns instead of PE matmuls + full-size
    ACT ln/exp); normalize via DVE reciprocal + broadcast multiply;
    DMA-transpose back. The per-(agent,quarter) broadcast matmuls read
    28-row slices of the transposed weights.
  - output bias evac batched across tile pairs ([2, BT] per 2 tiles).
  - reps>1 (timing builds) wrap the whole tile loop in a tc.For_i
    hardware loop, so the NEFF stays small and the repeat-slope timing
    signal can be made arbitrarily large.

The tile loop is software-pipelined over 3 stages (A: inputs/self/
enc/keys/logits; B: softmax; C: weighted vals + output) with the same
skew as before: tile t's softmax is emitted inside tile t+1's stage A,
its weighted phase two tiles later.
"""

import numpy as np
import ml_dtypes

B = 32768
NA = 8
A = NA - 1
OBS = 64
ACTD = 14
OTH_IN = 82
H_SELF = 64
H_OTH = 128
H2 = 64
HEADS = 4
AD = H_OTH // HEADS  # 32
NCORES = 8
BC = B // NCORES     # 4096 samples per core
BT = 512             # batch tile (free dim per matmul)
NT = BC // BT        # 8 tiles per core
X_IN = OBS + ACTD    # 78
A_SPLIT = 3          # agents emitted in stage-A front vs back
LROWS = 32           # logits rows (28 used: (h, a) = 7h + a; 4 pad)

BF16 = ml_dtypes.bfloat16

_CACHE = {}

# ---- tuning knobs ----------------------------------------------------
# engine for each PSUM->SBUF evacuation: "act" or "dve"
EVAC = {
    "x1": "act", "x2": "act", "sel": "act", "x3": "act",
    "enc": ["act", "act", "act", "act", "act", "act", "act"],
}
# engine for the 7 vals evacuations (relu+bias, PSUM->SBUF): "act" or "dve"
VEVAC = ["act", "act", "act", "act", "act", "dve", "dve"]
# engine for the transposed-softmax normalize multiplies: "dve" or "pool"
WNORM = "dve"


def _split_sync_waits(nc):
    """This walrus build rejects instructions carrying too many sem-wait
    conditions ("Too many sync wait commands"): 2 for compute instructions,
    1 for CTRL ops (Drain etc). Split extra waits onto preceding same-engine
    NOPs — engines execute their own stream in order, so a wait on an
    earlier NOP is equivalent."""
    import concourse.mybir as mybir

    n_added = 0
    for fn in nc.m.functions:
        for bb in fn.blocks:
            out = []
            for inst in bb.instructions:
                max_waits = 1
                si = inst.sync_info
                if si is not None and si.on_wait and len(si.on_wait) > max_waits:
                    waits = list(si.on_wait)
                    si.on_wait = waits[:max_waits]
                    rest = waits[max_waits:]
                    for k in range(0, len(rest), 1):
                        nop = mybir.InstNoOp(
                            name=f"{inst.name}-ws{k}", ins=[], outs=[],
                            bass_nofuse=True)
                        nop.engine = inst.engine
                        nop.sync_info = mybir.SyncInfo(
                            on_wait=[rest[k]], on_update=[])
                        out.append(nop)
                        n_added += 1
                out.append(inst)
            bb.instructions[:] = out
    return n_added


# Packed-constant layouts: (name, rows, cols). Offsets 4-col aligned.
_CONSTS_BF16 = [
    ("w1", 78, 64), ("w2", 64, 64), ("w3s", 64, 64), ("wq", 64, 128),
    ("we", 82, 7 * 128), ("wk", 128, 128), ("wv", 128, 128),
    ("w3o", 128, 64), ("wout", 64, 1), ("sind", 128, 7 * LROWS),
    ("wl", 28, 7 * 128),
]
_CONSTS_F32 = [
    ("b1", 64, 1), ("b2", 64, 1), ("be", 128, 7), ("bv", 128, 1),
    ("bout", 1, 1),
]


def _pack_layout(spec):
    off, w = {}, 0
    for name, rows, cols in spec:
        off[name] = w
        w += (cols + 3) // 4 * 4
    return off, w


def _const_view(spec, off, name):
    for n, rows, cols in spec:
        if n == name:
            return rows, off[name], off[name] + cols
    raise KeyError(name)


def _indicator_constants():
    """sind[j, a, r]: segred lhsT — agent a's head sums land in logits
    row r = 7h + a (h-major, rows 28..31 stay zero).
    wl2[32*(q%2)+r, a, j]: the broadcast lhsT wl[r, a, j] (wbc_a[j] =
    w[7*(j//32)+a]) replicated at partition offsets 0 and 32 so the
    per-quarter wbc matmuls read lhsT and rhs from the same legal base
    partition (AP bases are restricted to 0/32/64)."""
    sind = np.zeros((H_OTH, A, LROWS), dtype=BF16)
    for hd in range(H_OTH):
        for a in range(A):
            sind[hd, a, A * (hd // AD) + a] = 1.0
    wl = np.zeros((28, A, H_OTH), dtype=BF16)
    for p in range(H_OTH):
        for a in range(A):
            wl[A * (p // AD) + a, a, p] = 1.0
    return sind, wl


def _build_nc(reps=1):
    import concourse.bass as bass
    import concourse.mybir as mybir
    import concourse.tile as tile
    from contextlib import ExitStack

    dt = mybir.dt
    AF = mybir.ActivationFunctionType
    ALU = mybir.AluOpType

    nc = bass.Bass("TRN2", target_bir_lowering=False, debug=False)

    # ---- DRAM I/O ------------------------------------------------------
    # merged input: blocks 0..6 = other-agent inputs, block 7 rows
    # 0..77 = the self-branch input (padded to 82 rows)
    ot = nc.dram_tensor("ot", [OTH_IN, NA, BC], dt.bfloat16,
                        kind="ExternalInput")
    cb_off, cb_w = _pack_layout(_CONSTS_BF16)
    cf_off, cf_w = _pack_layout(_CONSTS_F32)
    cb = nc.dram_tensor("cb", [128, cb_w], dt.bfloat16, kind="ExternalInput")
    cf = nc.dram_tensor("cf", [128, cf_w], dt.float32, kind="ExternalInput")

    out_d = nc.dram_tensor("out", [1, BC], dt.float32, kind="ExternalOutput")

    with tile.TileContext(nc) as tc, ExitStack() as ctx:
        singles = ctx.enter_context(tc.tile_pool(name="singles", bufs=1))

        s_cb = singles.tile([128, cb_w], dt.bfloat16, name="s_cb")
        nc.sync.dma_start(out=s_cb, in_=cb.ap())
        s_cf = singles.tile([128, cf_w], dt.float32, name="s_cf")
        nc.sync.dma_start(out=s_cf, in_=cf.ap())

        def bslice(name, rows=None):
            r, c0, c1 = _const_view(_CONSTS_BF16, cb_off, name)
            return s_cb[: (rows or r), c0:c1]

        def fslice(name, rows=None):
            r, c0, c1 = _const_view(_CONSTS_F32, cf_off, name)
            return s_cf[: (rows or r), c0:c1]

        s_w1 = bslice("w1")
        s_w2 = bslice("w2")
        s_w3s = bslice("w3s")
        s_wq = bslice("wq")
        s_wk = bslice("wk")
        s_wv = bslice("wv")
        s_w3o = bslice("w3o")
        s_wout = bslice("wout")
        s_b1 = fslice("b1")
        s_b2 = fslice("b2")
        s_be = fslice("be")
        s_bv = fslice("bv")
        s_bout = fslice("bout")
        _, we0, _ = _const_view(_CONSTS_BF16, cb_off, "we")
        _, si0, _ = _const_view(_CONSTS_BF16, cb_off, "sind")
        _, wl0, _ = _const_view(_CONSTS_BF16, cb_off, "wl")

        def s_we(a):
            return s_cb[:OTH_IN, we0 + a * H_OTH: we0 + (a + 1) * H_OTH]

        def s_sind(a):
            return s_cb[:, si0 + a * LROWS: si0 + (a + 1) * LROWS]

        def s_wl(a):
            return s_cb[0:28, wl0 + a * H_OTH: wl0 + (a + 1) * H_OTH]

        # SBUF working pools
        p_ot = ctx.enter_context(tc.tile_pool(name="p_ot", bufs=3))
        p_act = ctx.enter_context(tc.tile_pool(name="p_act", bufs=5))
        p_enc = ctx.enter_context(tc.tile_pool(name="p_enc", bufs=4))
        p_prod = ctx.enter_context(tc.tile_pool(name="p_prod", bufs=3))
        p_sm = ctx.enter_context(tc.tile_pool(name="p_sm", bufs=2))
        p_eeT = ctx.enter_context(tc.tile_pool(name="p_eeT", bufs=2))
        p_z = ctx.enter_context(tc.tile_pool(name="p_z", bufs=2))
        p_pa = ctx.enter_context(tc.tile_pool(name="p_pa", bufs=4))

        outs_all = singles.tile([1, BC], dt.float32, name="outs_all")
        # transposed-softmax ping-pong buffers; pad columns (28..31 of
        # each 32-block) are zeroed once so the back-transpose never
        # carries garbage.
        # normalized-weight staging, ping-ponged by tile parity. One tile
        # PER BATCH QUARTER, its 28 weight rows at columns 0..27 (the rest
        # zero pad: the XBAR back-transpose needs a 128-wide source), so
        # every broadcast matmul reads operands at base partition 0 —
        # consecutive matmuls with differing operand base partitions hang
        # the hardware.
        wTs = [[singles.tile([128, 128], dt.bfloat16, name=f"wT{i}_{q}")
                for q in range(4)] for i in range(2)]
        wqs = [[singles.tile([128, 128], dt.bfloat16, name=f"wq{i}_{q}")
                for q in range(4)] for i in range(2)]
        for row in wTs:
            for wt in row:
                nc.vector.memset(wt, 0.0)

        # PSUM pools (8 banks total)
        psA = ctx.enter_context(tc.tile_pool(name="psA", bufs=1, space="PSUM"))
        psE = ctx.enter_context(tc.tile_pool(name="psE", bufs=2, space="PSUM"))
        psK = ctx.enter_context(tc.tile_pool(name="psK", bufs=2, space="PSUM"))
        psW = ctx.enter_context(tc.tile_pool(name="psW", bufs=2, space="PSUM"))
        psLX = ctx.enter_context(tc.tile_pool(name="psLX", bufs=1,
                                              space="PSUM"))

        # one shared bank: logits at partitions 0..31, x3 at 64..127
        lx_t = psLX.tile([128, BT], dt.float32, name="lx_t")
        x3o_t = lx_t[64:128, :]

        def evac(engine, out, in_, bias=None, relu=False):
            """PSUM->SBUF evacuation on the chosen engine."""
            if engine == "act":
                if relu:
                    nc.scalar.activation(out, in_, AF.Relu,
                                         bias=bias if bias is not None else 0.0)
                elif bias is not None:
                    nc.scalar.activation(out, in_, AF.Identity, bias=bias)
                else:
                    nc.scalar.activation(out, in_, AF.Copy)
            else:
                if relu:
                    nc.vector.tensor_scalar(
                        out=out, in0=in_, scalar1=bias, scalar2=0.0,
                        op0=ALU.add, op1=ALU.max)
                elif bias is not None:
                    nc.vector.tensor_scalar(
                        out=out, in0=in_, scalar1=bias, scalar2=None,
                        op0=ALU.add)
                else:
                    nc.vector.tensor_copy(out, in_)

        def stage_a(t):
            """Inputs, self branch, the agent enc/keys/logits loop.

            Cross-engine latency hiding: the first enc matmul fills the
            x1-evacuation wait; inside the agent loop the next agent's enc
            matmul is emitted one step ahead of its use, and each segred
            matmul is delayed one step so its DVE product has time to
            finish before the PE reaches it."""
            b0 = (t % NT) * BT
            ots = p_ot.tile([OTH_IN, NA, BT], dt.bfloat16, tag="ots")
            nc.sync.dma_start(out=ots, in_=ot.ap()[:, :, b0:b0 + BT])
            xts = ots[:X_IN, A, :]

            x1p = psA.tile([H_SELF, BT], dt.float32, tag="m", name="x1p")
            nc.tensor.matmul(x1p, s_w1, xts, start=True, stop=True)
            encps = [psE.tile([H_OTH, BT], dt.float32, tag="m", name="encp")]
            nc.tensor.matmul(encps[0], s_we(0), ots[:, 0, :],
                             start=True, stop=True)
            x1 = p_act.tile([H_SELF, BT], dt.bfloat16, tag="x1")
            evac(EVAC["x1"], x1, x1p, bias=s_b1, relu=True)

            selp = psA.tile([H_OTH, BT], dt.float32, tag="m", name="selp")
            nc.tensor.matmul(selp, s_wq, x1, start=True, stop=True)

            lp = lx_t[0:LROWS, :]
            enc = p_enc.tile([H_OTH, A, BT], dt.bfloat16, tag="enc")

            st = {"lp": lp, "enc": enc, "b0": b0,
                  "ots": ots, "t": t % NT}
            evac(EVAC["enc"][0], enc[:, 0, :], encps[0],
                 bias=s_be[:, 0:1], relu=True)
            sel = p_act.tile([H_OTH, BT], dt.bfloat16, tag="sel")
            evac(EVAC["sel"], sel, selp)

            prods = []
            for a in range(A):
                if len(prods) >= 2:
                    # segred delayed two agents behind its product
                    nc.tensor.matmul(lp, s_sind(a - 2), prods[a - 2],
                                     start=(a == 2), stop=False,
                                     skip_group_check=True)
                if a + 1 < A:
                    encps.append(psE.tile([H_OTH, BT], dt.float32,
                                          tag="m", name="encp"))
                    nc.tensor.matmul(encps[a + 1], s_we(a + 1),
                                     ots[:, a + 1, :], start=True, stop=True)
                    evac(EVAC["enc"][a + 1], enc[:, a + 1, :], encps[a + 1],
                         bias=s_be[:, a + 1:a + 2], relu=True)
                keysp = psK.tile([H_OTH, BT], dt.float32, tag="kv",
                                 name="keysp")
                nc.tensor.matmul(keysp, s_wk, enc[:, a, :],
                                 start=True, stop=True)
                prod = p_prod.tile([H_OTH, BT], dt.bfloat16, tag="prod")
                nc.vector.tensor_mul(out=prod, in0=sel, in1=keysp)
                prods.append(prod)
            nc.tensor.matmul(lp, s_sind(A - 2), prods[A - 2],
                             start=False, stop=False, skip_group_check=True)
            nc.tensor.matmul(lp, s_sind(A - 1), prods[A - 1],
                             start=False, stop=True, skip_group_check=True)

            # deferred second self layer: x2 is needed only by stage C
            x2p = psA.tile([H_SELF, BT], dt.float32, tag="m", name="x2p")
            nc.tensor.matmul(x2p, s_w2, x1, start=True, stop=True)
            x2 = p_act.tile([H_SELF, BT], dt.bfloat16, tag="x2")
            evac(EVAC["x2"], x2, x2p, bias=s_b2, relu=True)
            st["x2"] = x2
            return st

        def stage_b1a(st):
            """Softmax head: exp, then transpose to batch-major."""
            lp = st["lp"]
            ee = p_sm.tile([LROWS, BT], dt.bfloat16, tag="ee")
            nc.scalar.activation(ee, lp, AF.Exp)
            eeT = p_eeT.tile([128, 4, LROWS], dt.bfloat16, tag="eeT")
            nc.sync.dma_start_transpose(eeT, ee)
            st["eeT"] = eeT

        def stage_b1(st):
            """Softmax tail in transposed space: segment sums via strided
            reduce, reciprocal, normalize, transpose back."""
            eeT = st["eeT"]
            idx = st["t"] % 2
            zt = p_z.tile([128, 4, HEADS], dt.float32, tag="z", name="zt")
            nc.vector.tensor_reduce(
                out=zt, in_=eeT[:, :, 0:28].rearrange(
                    "p q (h a) -> p q h a", h=HEADS),
                op=ALU.add, axis=mybir.AxisListType.X)
            rz = p_z.tile([128, 4, HEADS], dt.float32, tag="z", name="rz")
            nc.vector.reciprocal(rz, zt)
            eng = nc.gpsimd if WNORM == "pool" else nc.vector
            for q in range(4):
                wt = wTs[idx][q]
                eng.tensor_tensor(
                    out=wt[:, 0:28].rearrange("p (h a) -> p h a", h=HEADS),
                    in0=eeT[:, q, 0:28].rearrange("p (h a) -> p h a", h=HEADS),
                    in1=rz[:, q, :].unsqueeze(2).broadcast_to(
                        [128, HEADS, A]),
                    op=ALU.mult)
                nc.sync.dma_start_transpose(wqs[idx][q], wt)
            st["wq"] = wqs[idx]

        def stage_b2(st):
            """Weighted values: per-agent broadcast matmuls (by batch
            quarter), late Wv matmul, fused (vals+bv)*wbc, relu, w3o
            accumulation."""
            enc, wq_ = st["enc"], st["wq"]
            x3p = x3o_t
            nc.tensor.matmul(x3p, s_w3s, st["x2"], start=True, stop=False,
                             skip_group_check=True)
            par_prev = None
            for a in range(A):
                wbcp = psW.tile([H_OTH, BT], dt.float32, tag="wb",
                                name="wbcp")
                for q in range(4):
                    nc.tensor.matmul(
                        wbcp[:, q * 128:(q + 1) * 128], s_wl(a),
                        wq_[q][0:28, 0:128],
                        start=True, stop=True, skip_group_check=True)
                valsp = psK.tile([H_OTH, BT], dt.float32, tag="kv",
                                 name="valsp")
                nc.tensor.matmul(valsp, s_wv, enc[:, a, :],
                                 start=True, stop=True)
                if par_prev is not None:
                    nc.tensor.matmul(x3p, s_w3o, par_prev, start=False,
                                     stop=False, skip_group_check=True)
                # vals = relu(valsp + bv) to SBUF (DVE ops may read only
                # one PSUM operand, so the product below needs vals there)
                vals = p_pa.tile([H_OTH, BT], dt.bfloat16, tag="pa")
                evac(VEVAC[a], vals, valsp, bias=s_bv, relu=True)
                par = p_pa.tile([H_OTH, BT], dt.bfloat16, tag="pa")
                nc.vector.tensor_mul(out=par, in0=vals, in1=wbcp)
                par_prev = par
            nc.tensor.matmul(x3p, s_w3o, par_prev, start=False,
                             stop=True, skip_group_check=True)
            st["x3p"] = x3p

        def stage_b3a(st):
            """Relu-evacuate x3; the wout matmul + bias run one iteration
            later (stage_b3b) so the ACT->PE->ACT chain never stalls."""
            x3 = p_act.tile([H2, BT], dt.bfloat16, tag="x3s")
            evac(EVAC["x3"], x3, st["x3p"], relu=True)
            st["x3"] = x3

        def stage_b3b(st):
            outp = psA.tile([1, BT], dt.float32, tag="m", name="outp")
            nc.tensor.matmul(outp, s_wout, st["x3"], start=True, stop=True)
            b0 = st["b0"]
            nc.scalar.activation(outs_all[:, b0:b0 + BT], outp, AF.Identity,
                                 bias=s_bout)

        import os
        KLEVEL = int(os.environ.get("KLEVEL", "5"))

        def emit_body():
            prev = prev2 = prev3 = prev4 = prev5 = None
            for t in range(NT):
                if prev is not None and KLEVEL >= 2:
                    stage_b1a(prev)
                stf = stage_a(t)
                if prev2 is not None and KLEVEL >= 3:
                    stage_b1(prev2)
                if prev5 is not None and KLEVEL >= 5:
                    stage_b3b(prev5)
                if prev4 is not None and KLEVEL >= 5:
                    stage_b3a(prev4)
                if prev3 is not None and KLEVEL >= 4:
                    stage_b2(prev3)
                prev5 = prev4
                prev4 = prev3
                prev3 = prev2
                prev2 = prev
                prev = stf
            if KLEVEL < 5:
                nc.vector.memset(outs_all, 0.0)
                return
            stage_b1a(prev)
            stage_b1(prev2)
            stage_b3b(prev5)
            stage_b3a(prev4)
            stage_b2(prev3)
            stage_b1(prev)
            stage_b3b(prev4)
            stage_b3a(prev3)
            stage_b2(prev2)
            stage_b3b(prev3)
            stage_b3a(prev2)
            stage_b2(prev)
            stage_b3b(prev2)
            stage_b3a(prev)
            stage_b3b(prev)

        if reps == 1:
            emit_body()
        else:
            with tc.For_i(0, reps):
                emit_body()

        nc.sync.dma_start(out=out_d.ap(), in_=outs_all)

    _split_sync_waits(nc)
    return nc


def _prep_inputs(state_one, act_one, state_others, act_others,
                 W1, b1, W2, b2, w3_self, We, be,
                 Wk, Wq, Wv, bv, w3_others, Wout, bout):
    """Host-side sharding + layout transforms. Returns per-core in_maps."""
    scale = 1.0 / np.sqrt(np.float32(AD))

    xt_full = np.concatenate([state_one, act_one], axis=1).T  # [78, B]
    inps = np.concatenate([state_others, act_others], axis=2)  # [A, B, 82]
    ot_full = np.zeros((OTH_IN, NA, B), dtype=BF16)
    ot_full[:, :A, :] = np.transpose(inps, (2, 0, 1))
    ot_full[:X_IN, A, :] = xt_full

    def headcat(wm):  # [H, J, AD] -> [J, H*AD]
        return np.ascontiguousarray(
            np.transpose(np.asarray(wm, np.float32), (1, 0, 2))
            .reshape(wm.shape[1], HEADS * AD))

    sind, wl = _indicator_constants()

    vals_bf16 = {
        "w1": np.asarray(W1, np.float32).astype(BF16),
        "w2": np.asarray(W2, np.float32).astype(BF16),
        "w3s": np.asarray(w3_self, np.float32).astype(BF16),
        "wq": (headcat(Wq) * scale).astype(BF16),
        "we": np.ascontiguousarray(
            np.transpose(np.asarray(We, np.float32), (1, 0, 2))
            .reshape(OTH_IN, A * H_OTH)).astype(BF16),
        "wk": headcat(Wk).astype(BF16),
        "wv": headcat(Wv).astype(BF16),
        "w3o": np.asarray(w3_others, np.float32).astype(BF16),
        "wout": np.asarray(Wout, np.float32).astype(BF16),
        "sind": sind.reshape(H_OTH, A * LROWS),
        "wl": wl.reshape(28, A * H_OTH),
    }
    vals_f32 = {
        "b1": np.asarray(b1, np.float32).reshape(H_SELF, 1),
        "b2": np.asarray(b2, np.float32).reshape(H_SELF, 1),
        "be": np.ascontiguousarray(np.asarray(be, np.float32).T),
        "bv": np.asarray(bv, np.float32).reshape(HEADS * AD, 1),
        "bout": np.asarray(bout, np.float32).reshape(1, 1),
    }

    def pack(spec, values, dtype):
        off, width = _pack_layout(spec)
        arr = np.zeros((128, width), dtype=dtype)
        for name, rows, cols in spec:
            v = values[name]
            assert v.shape == (rows, cols), (name, v.shape, rows, cols)
            arr[:rows, off[name]:off[name] + cols] = v
        return arr

    cb = pack(_CONSTS_BF16, vals_bf16, BF16)
    cf = pack(_CONSTS_F32, vals_f32, np.float32)

    in_maps = []
    for c in range(NCORES):
        sl = slice(c * BC, (c + 1) * BC)
        m = {"cb": cb, "cf": cf,
             "ot": np.ascontiguousarray(ot_full[:, :, sl])}
        in_maps.append(m)
    return in_maps


def get_nc(reps=1):
    key = ("nc", reps)
    if key not in _CACHE:
        _CACHE[key] = _build_nc(reps)
    return _CACHE[key]


def kernel(**inputs) -> np.ndarray:
    from concourse.bass_utils import run_bass_kernel_spmd

    nc = get_nc()
    in_maps = _prep_inputs(**inputs)
    res = run_bass_kernel_spmd(nc, in_maps, core_ids=list(range(NCORES)))
    out = np.concatenate(
        [np.asarray(res.results[c]["out"], np.float32).reshape(BC, 1)
         for c in range(NCORES)], axis=0)
    return out


# revision 45
# speedup vs baseline: 1.0113x; 1.0113x over previous
"""Trainium2 Bass kernel for the multi-agent attention critic.

Strategy: data-parallel over the batch dim across 8 NeuronCores
(4096 samples/core). Feature-major ([feature, batch] tiles, batch on
the free dim) in bf16 with fp32 PSUM accumulation.

v2 restructure vs the earlier baseline:
  - vals are computed LATE (in the weighted phase): the Wv matmul output
    stays in PSUM and a fused DVE scalar_tensor_tensor computes
    (valsp + bv) * wbc in one op; the relu commutes with the softmax
    weight (w >= 0) so a cheap all-bf16 4x-mode relu follows. This
    removes all seven vals ACT evacuations per tile.
  - softmax runs in transposed (batch-on-partitions) space: ee=exp(L)
    [32, BT] -> DMA-transpose -> [128, 4, 32]; segment sums become a
    strided DVE tensor_reduce (177 ns instead of PE matmuls + full-size
    ACT ln/exp); normalize via DVE reciprocal + broadcast multiply;
    DMA-transpose back. The per-(agent,quarter) broadcast matmuls read
    28-row slices of the transposed weights.
  - output bias evac batched across tile pairs ([2, BT] per 2 tiles).
  - reps>1 (timing builds) wrap the whole tile loop in a tc.For_i
    hardware loop, so the NEFF stays small and the repeat-slope timing
    signal can be made arbitrarily large.

The tile loop is software-pipelined over 3 stages (A: inputs/self/
enc/keys/logits; B: softmax; C: weighted vals + output) with the same
skew as before: tile t's softmax is emitted inside tile t+1's stage A,
its weighted phase two tiles later.
"""

import numpy as np
import ml_dtypes

B = 32768
NA = 8
A = NA - 1
OBS = 64
ACTD = 14
OTH_IN = 82
H_SELF = 64
H_OTH = 128
H2 = 64
HEADS = 4
AD = H_OTH // HEADS  # 32
NCORES = 8
BC = B // NCORES     # 4096 samples per core
BT = 512             # batch tile (free dim per matmul)
NT = BC // BT        # 8 tiles per core
X_IN = OBS + ACTD    # 78
A_SPLIT = 3          # agents emitted in stage-A front vs back
LROWS = 32           # logits rows (28 used: (h, a) = 7h + a; 4 pad)

BF16 = ml_dtypes.bfloat16

_CACHE = {}

# ---- tuning knobs ----------------------------------------------------
# engine for each PSUM->SBUF evacuation: "act" or "dve"
EVAC = {
    "x1": "act", "x2": "act", "sel": "act", "x3": "act",
    "enc": ["act", "act", "act", "act", "act", "act", "act"],
}
# engine for the 7 vals evacuations (relu+bias, PSUM->SBUF): "act" or "dve"
VEVAC = ["act", "act", "act", "act", "act", "act", "dve"]
# engine for the transposed-softmax normalize multiplies: "dve" or "pool"
WNORM = "pool"


def _split_sync_waits(nc):
    """This walrus build rejects instructions carrying too many sem-wait
    conditions ("Too many sync wait commands"): 2 for compute instructions,
    1 for CTRL ops (Drain etc). Split extra waits onto preceding same-engine
    NOPs — engines execute their own stream in order, so a wait on an
    earlier NOP is equivalent."""
    import concourse.mybir as mybir

    n_added = 0
    for fn in nc.m.functions:
        for bb in fn.blocks:
            out = []
            for inst in bb.instructions:
                max_waits = 1
                si = inst.sync_info
                if si is not None and si.on_wait and len(si.on_wait) > max_waits:
                    waits = list(si.on_wait)
                    si.on_wait = waits[:max_waits]
                    rest = waits[max_waits:]
                    for k in range(0, len(rest), 1):
                        nop = mybir.InstNoOp(
                            name=f"{inst.name}-ws{k}", ins=[], outs=[],
                            bass_nofuse=True)
                        nop.engine = inst.engine
                        nop.sync_info = mybir.SyncInfo(
                            on_wait=[rest[k]], on_update=[])
                        out.append(nop)
                        n_added += 1
                out.append(inst)
            bb.instructions[:] = out
    return n_added


# Packed-constant layouts: (name, rows, cols). Offsets 4-col aligned.
_CONSTS_BF16 = [
    ("w1", 78, 64), ("w2", 64, 64), ("w3s", 64, 64), ("wq", 64, 128),
    ("we", 82, 7 * 128), ("wk", 128, 128), ("wv", 128, 128),
    ("w3o", 128, 64), ("wout", 64, 1), ("sind", 128, 7 * LROWS),
    ("wl", 28, 7 * 128),
]
_CONSTS_F32 = [
    ("b1", 64, 1), ("b2", 64, 1), ("be", 128, 7), ("bv", 128, 1),
    ("bout", 1, 1),
]


def _pack_layout(spec):
    off, w = {}, 0
    for name, rows, cols in spec:
        off[name] = w
        w += (cols + 3) // 4 * 4
    return off, w


def _const_view(spec, off, name):
    for n, rows, cols in spec:
        if n == name:
            return rows, off[name], off[name] + cols
    raise KeyError(name)


def _indicator_constants():
    """sind[j, a, r]: segred lhsT — agent a's head sums land in logits
    row r = 7h + a (h-major, rows 28..31 stay zero).
    wl2[32*(q%2)+r, a, j]: the broadcast lhsT wl[r, a, j] (wbc_a[j] =
    w[7*(j//32)+a]) replicated at partition offsets 0 and 32 so the
    per-quarter wbc matmuls read lhsT and rhs from the same legal base
    partition (AP bases are restricted to 0/32/64)."""
    sind = np.zeros((H_OTH, A, LROWS), dtype=BF16)
    for hd in range(H_OTH):
        for a in range(A):
            sind[hd, a, A * (hd // AD) + a] = 1.0
    wl = np.zeros((28, A, H_OTH), dtype=BF16)
    for p in range(H_OTH):
        for a in range(A):
            wl[A * (p // AD) + a, a, p] = 1.0
    return sind, wl


def _build_nc(reps=1):
    import concourse.bass as bass
    import concourse.mybir as mybir
    import concourse.tile as tile
    from contextlib import ExitStack

    dt = mybir.dt
    AF = mybir.ActivationFunctionType
    ALU = mybir.AluOpType

    nc = bass.Bass("TRN2", target_bir_lowering=False, debug=False)

    # ---- DRAM I/O ------------------------------------------------------
    # merged input: blocks 0..6 = other-agent inputs, block 7 rows
    # 0..77 = the self-branch input (padded to 82 rows)
    ot = nc.dram_tensor("ot", [OTH_IN, NA, BC], dt.bfloat16,
                        kind="ExternalInput")
    cb_off, cb_w = _pack_layout(_CONSTS_BF16)
    cf_off, cf_w = _pack_layout(_CONSTS_F32)
    cb = nc.dram_tensor("cb", [128, cb_w], dt.bfloat16, kind="ExternalInput")
    cf = nc.dram_tensor("cf", [128, cf_w], dt.float32, kind="ExternalInput")

    out_d = nc.dram_tensor("out", [1, BC], dt.float32, kind="ExternalOutput")

    with tile.TileContext(nc) as tc, ExitStack() as ctx:
        singles = ctx.enter_context(tc.tile_pool(name="singles", bufs=1))

        s_cb = singles.tile([128, cb_w], dt.bfloat16, name="s_cb")
        nc.sync.dma_start(out=s_cb, in_=cb.ap())
        s_cf = singles.tile([128, cf_w], dt.float32, name="s_cf")
        nc.sync.dma_start(out=s_cf, in_=cf.ap())

        def bslice(name, rows=None):
            r, c0, c1 = _const_view(_CONSTS_BF16, cb_off, name)
            return s_cb[: (rows or r), c0:c1]

        def fslice(name, rows=None):
            r, c0, c1 = _const_view(_CONSTS_F32, cf_off, name)
            return s_cf[: (rows or r), c0:c1]

        s_w1 = bslice("w1")
        s_w2 = bslice("w2")
        s_w3s = bslice("w3s")
        s_wq = bslice("wq")
        s_wk = bslice("wk")
        s_wv = bslice("wv")
        s_w3o = bslice("w3o")
        s_wout = bslice("wout")
        s_b1 = fslice("b1")
        s_b2 = fslice("b2")
        s_be = fslice("be")
        s_bv = fslice("bv")
        s_bout = fslice("bout")
        _, we0, _ = _const_view(_CONSTS_BF16, cb_off, "we")
        _, si0, _ = _const_view(_CONSTS_BF16, cb_off, "sind")
        _, wl0, _ = _const_view(_CONSTS_BF16, cb_off, "wl")

        def s_we(a):
            return s_cb[:OTH_IN, we0 + a * H_OTH: we0 + (a + 1) * H_OTH]

        def s_sind(a):
            return s_cb[:, si0 + a * LROWS: si0 + (a + 1) * LROWS]

        def s_wl(a):
            return s_cb[0:28, wl0 + a * H_OTH: wl0 + (a + 1) * H_OTH]

        # SBUF working pools
        p_ot = ctx.enter_context(tc.tile_pool(name="p_ot", bufs=3))
        p_act = ctx.enter_context(tc.tile_pool(name="p_act", bufs=5))
        p_enc = ctx.enter_context(tc.tile_pool(name="p_enc", bufs=4))
        p_prod = ctx.enter_context(tc.tile_pool(name="p_prod", bufs=3))
        p_sm = ctx.enter_context(tc.tile_pool(name="p_sm", bufs=2))
        p_eeT = ctx.enter_context(tc.tile_pool(name="p_eeT", bufs=2))
        p_z = ctx.enter_context(tc.tile_pool(name="p_z", bufs=2))
        p_pa = ctx.enter_context(tc.tile_pool(name="p_pa", bufs=4))

        outs_all = singles.tile([1, BC], dt.float32, name="outs_all")
        # transposed-softmax ping-pong buffers; pad columns (28..31 of
        # each 32-block) are zeroed once so the back-transpose never
        # carries garbage.
        # normalized-weight staging, ping-ponged by tile parity. One tile
        # PER BATCH QUARTER, its 28 weight rows at columns 0..27 (the rest
        # zero pad: the XBAR back-transpose needs a 128-wide source), so
        # every broadcast matmul reads operands at base partition 0 —
        # consecutive matmuls with differing operand base partitions hang
        # the hardware.
        wTs = [[singles.tile([128, 128], dt.bfloat16, name=f"wT{i}_{q}")
                for q in range(4)] for i in range(2)]
        wqs = [[singles.tile([128, 128], dt.bfloat16, name=f"wq{i}_{q}")
                for q in range(4)] for i in range(2)]
        for row in wTs:
            for wt in row:
                nc.vector.memset(wt, 0.0)

        # PSUM pools (8 banks total)
        psA = ctx.enter_context(tc.tile_pool(name="psA", bufs=1, space="PSUM"))
        psE = ctx.enter_context(tc.tile_pool(name="psE", bufs=2, space="PSUM"))
        psK = ctx.enter_context(tc.tile_pool(name="psK", bufs=2, space="PSUM"))
        psW = ctx.enter_context(tc.tile_pool(name="psW", bufs=2, space="PSUM"))
        psLX = ctx.enter_context(tc.tile_pool(name="psLX", bufs=1,
                                              space="PSUM"))

        # one shared bank: logits at partitions 0..31, x3 at 64..127
        lx_t = psLX.tile([128, BT], dt.float32, name="lx_t")
        x3o_t = lx_t[64:128, :]

        def evac(engine, out, in_, bias=None, relu=False):
            """PSUM->SBUF evacuation on the chosen engine."""
            if engine == "act":
                if relu:
                    nc.scalar.activation(out, in_, AF.Relu,
                                         bias=bias if bias is not None else 0.0)
                elif bias is not None:
                    nc.scalar.activation(out, in_, AF.Identity, bias=bias)
                else:
                    nc.scalar.activation(out, in_, AF.Copy)
            else:
                if relu:
                    nc.vector.tensor_scalar(
                        out=out, in0=in_, scalar1=bias, scalar2=0.0,
                        op0=ALU.add, op1=ALU.max)
                elif bias is not None:
                    nc.vector.tensor_scalar(
                        out=out, in0=in_, scalar1=bias, scalar2=None,
                        op0=ALU.add)
                else:
                    nc.vector.tensor_copy(out, in_)

        def stage_a(t):
            """Inputs, self branch, the agent enc/keys/logits loop.

            Cross-engine latency hiding: the first enc matmul fills the
            x1-evacuation wait; inside the agent loop the next agent's enc
            matmul is emitted one step ahead of its use, and each segred
            matmul is delayed one step so its DVE product has time to
            finish before the PE reaches it."""
            b0 = (t % NT) * BT
            ots = p_ot.tile([OTH_IN, NA, BT], dt.bfloat16, tag="ots")
            nc.sync.dma_start(out=ots, in_=ot.ap()[:, :, b0:b0 + BT])
            xts = ots[:X_IN, A, :]

            x1p = psA.tile([H_SELF, BT], dt.float32, tag="m", name="x1p")
            nc.tensor.matmul(x1p, s_w1, xts, start=True, stop=True)
            encps = [psE.tile([H_OTH, BT], dt.float32, tag="m", name="encp")]
            nc.tensor.matmul(encps[0], s_we(0), ots[:, 0, :],
                             start=True, stop=True)
            x1 = p_act.tile([H_SELF, BT], dt.bfloat16, tag="x1")
            evac(EVAC["x1"], x1, x1p, bias=s_b1, relu=True)

            selp = psA.tile([H_OTH, BT], dt.float32, tag="m", name="selp")
            nc.tensor.matmul(selp, s_wq, x1, start=True, stop=True)

            lp = lx_t[0:LROWS, :]
            enc = p_enc.tile([H_OTH, A, BT], dt.bfloat16, tag="enc")

            st = {"lp": lp, "enc": enc, "b0": b0,
                  "ots": ots, "t": t % NT}
            evac(EVAC["enc"][0], enc[:, 0, :], encps[0],
                 bias=s_be[:, 0:1], relu=True)
            sel = p_act.tile([H_OTH, BT], dt.bfloat16, tag="sel")
            evac(EVAC["sel"], sel, selp)

            prods = []
            for a in range(A):
                if len(prods) >= 2:
                    # segred delayed two agents behind its product
                    nc.tensor.matmul(lp, s_sind(a - 2), prods[a - 2],
                                     start=(a == 2), stop=False,
                                     skip_group_check=True)
                if a + 1 < A:
                    encps.append(psE.tile([H_OTH, BT], dt.float32,
                                          tag="m", name="encp"))
                    nc.tensor.matmul(encps[a + 1], s_we(a + 1),
                                     ots[:, a + 1, :], start=True, stop=True)
                    evac(EVAC["enc"][a + 1], enc[:, a + 1, :], encps[a + 1],
                         bias=s_be[:, a + 1:a + 2], relu=True)
                keysp = psK.tile([H_OTH, BT], dt.float32, tag="kv",
                                 name="keysp")
                nc.tensor.matmul(keysp, s_wk, enc[:, a, :],
                                 start=True, stop=True)
                prod = p_prod.tile([H_OTH, BT], dt.bfloat16, tag="prod")
                nc.vector.tensor_mul(out=prod, in0=sel, in1=keysp)
                prods.append(prod)
            nc.tensor.matmul(lp, s_sind(A - 2), prods[A - 2],
                             start=False, stop=False, skip_group_check=True)
            nc.tensor.matmul(lp, s_sind(A - 1), prods[A - 1],
                             start=False, stop=True, skip_group_check=True)

            # deferred second self layer: x2 is needed only by stage C
            x2p = psA.tile([H_SELF, BT], dt.float32, tag="m", name="x2p")
            nc.tensor.matmul(x2p, s_w2, x1, start=True, stop=True)
            x2 = p_act.tile([H_SELF, BT], dt.bfloat16, tag="x2")
            evac(EVAC["x2"], x2, x2p, bias=s_b2, relu=True)
            st["x2"] = x2
            return st

        def stage_b1a(st):
            """Softmax head: exp, then transpose to batch-major."""
            lp = st["lp"]
            ee = p_sm.tile([LROWS, BT], dt.bfloat16, tag="ee")
            nc.scalar.activation(ee, lp, AF.Exp)
            eeT = p_eeT.tile([128, 4, LROWS], dt.bfloat16, tag="eeT")
            nc.sync.dma_start_transpose(eeT, ee)
            st["eeT"] = eeT

        def stage_b1(st):
            """Softmax tail in transposed space: segment sums via strided
            reduce, reciprocal, normalize, transpose back."""
            eeT = st["eeT"]
            idx = st["t"] % 2
            zt = p_z.tile([128, 4, HEADS], dt.float32, tag="z", name="zt")
            nc.vector.tensor_reduce(
                out=zt, in_=eeT[:, :, 0:28].rearrange(
                    "p q (h a) -> p q h a", h=HEADS),
                op=ALU.add, axis=mybir.AxisListType.X)
            rz = p_z.tile([128, 4, HEADS], dt.float32, tag="z", name="rz")
            nc.vector.reciprocal(rz, zt)
            eng = nc.gpsimd if WNORM == "pool" else nc.vector
            for q in range(4):
                wt = wTs[idx][q]
                eng.tensor_tensor(
                    out=wt[:, 0:28].rearrange("p (h a) -> p h a", h=HEADS),
                    in0=eeT[:, q, 0:28].rearrange("p (h a) -> p h a", h=HEADS),
                    in1=rz[:, q, :].unsqueeze(2).broadcast_to(
                        [128, HEADS, A]),
                    op=ALU.mult)
                nc.sync.dma_start_transpose(wqs[idx][q], wt)
            st["wq"] = wqs[idx]

        def stage_b2(st):
            """Weighted values: per-agent broadcast matmuls (by batch
            quarter), late Wv matmul, fused (vals+bv)*wbc, relu, w3o
            accumulation."""
            enc, wq_ = st["enc"], st["wq"]
            x3p = x3o_t
            nc.tensor.matmul(x3p, s_w3s, st["x2"], start=True, stop=False,
                             skip_group_check=True)
            par_prev = None
            for a in range(A):
                wbcp = psW.tile([H_OTH, BT], dt.float32, tag="wb",
                                name="wbcp")
                for q in range(4):
                    nc.tensor.matmul(
                        wbcp[:, q * 128:(q + 1) * 128], s_wl(a),
                        wq_[q][0:28, 0:128],
                        start=True, stop=True, skip_group_check=True)
                valsp = psK.tile([H_OTH, BT], dt.float32, tag="kv",
                                 name="valsp")
                nc.tensor.matmul(valsp, s_wv, enc[:, a, :],
                                 start=True, stop=True)
                if par_prev is not None:
                    nc.tensor.matmul(x3p, s_w3o, par_prev, start=False,
                                     stop=False, skip_group_check=True)
                # vals = relu(valsp + bv) to SBUF (DVE ops may read only
                # one PSUM operand, so the product below needs vals there)
                vals = p_pa.tile([H_OTH, BT], dt.bfloat16, tag="pa")
                evac(VEVAC[a], vals, valsp, bias=s_bv, relu=True)
                par = p_pa.tile([H_OTH, BT], dt.bfloat16, tag="pa")
                nc.vector.tensor_mul(out=par, in0=vals, in1=wbcp)
                par_prev = par
            nc.tensor.matmul(x3p, s_w3o, par_prev, start=False,
                             stop=True, skip_group_check=True)
            st["x3p"] = x3p

        def stage_b3a(st):
            """Relu-evacuate x3; the wout matmul + bias run one iteration
            later (stage_b3b) so the ACT->PE->ACT chain never stalls."""
            x3 = p_act.tile([H2, BT], dt.bfloat16, tag="x3s")
            evac(EVAC["x3"], x3, st["x3p"], relu=True)
            st["x3"] = x3

        def stage_b3b(st):
            outp = psA.tile([1, BT], dt.float32, tag="m", name="outp")
            nc.tensor.matmul(outp, s_wout, st["x3"], start=True, stop=True)
            b0 = st["b0"]
            nc.scalar.activation(outs_all[:, b0:b0 + BT], outp, AF.Identity,
                                 bias=s_bout)

        import os
        KLEVEL = int(os.environ.get("KLEVEL", "5"))

        def emit_body():
            prev = prev2 = prev3 = prev4 = prev5 = None
            for t in range(NT):
                if prev is not None and KLEVEL >= 2:
                    stage_b1a(prev)
                stf = stage_a(t)
                if prev2 is not None and KLEVEL >= 3:
                    stage_b1(prev2)
                if prev5 is not None and KLEVEL >= 5:
                    stage_b3b(prev5)
                if prev4 is not None and KLEVEL >= 5:
                    stage_b3a(prev4)
                if prev3 is not None and KLEVEL >= 4:
                    stage_b2(prev3)
                prev5 = prev4
                prev4 = prev3
                prev3 = prev2
                prev2 = prev
                prev = stf
            if KLEVEL < 5:
                nc.vector.memset(outs_all, 0.0)
                return
            stage_b1a(prev)
            stage_b1(prev2)
            stage_b3b(prev5)
            stage_b3a(prev4)
            stage_b2(prev3)
            stage_b1(prev)
            stage_b3b(prev4)
            stage_b3a(prev3)
            stage_b2(prev2)
            stage_b3b(prev3)
            stage_b3a(prev2)
            stage_b2(prev)
            stage_b3b(prev2)
            stage_b3a(prev)
            stage_b3b(prev)

        if reps == 1:
            emit_body()
        else:
            with tc.For_i(0, reps):
                emit_body()

        nc.sync.dma_start(out=out_d.ap(), in_=outs_all)

    _split_sync_waits(nc)
    return nc


def _prep_inputs(state_one, act_one, state_others, act_others,
                 W1, b1, W2, b2, w3_self, We, be,
                 Wk, Wq, Wv, bv, w3_others, Wout, bout):
    """Host-side sharding + layout transforms. Returns per-core in_maps."""
    scale = 1.0 / np.sqrt(np.float32(AD))

    xt_full = np.concatenate([state_one, act_one], axis=1).T  # [78, B]
    inps = np.concatenate([state_others, act_others], axis=2)  # [A, B, 82]
    ot_full = np.zeros((OTH_IN, NA, B), dtype=BF16)
    ot_full[:, :A, :] = np.transpose(inps, (2, 0, 1))
    ot_full[:X_IN, A, :] = xt_full

    def headcat(wm):  # [H, J, AD] -> [J, H*AD]
        return np.ascontiguousarray(
            np.transpose(np.asarray(wm, np.float32), (1, 0, 2))
            .reshape(wm.shape[1], HEADS * AD))

    sind, wl = _indicator_constants()

    vals_bf16 = {
        "w1": np.asarray(W1, np.float32).astype(BF16),
        "w2": np.asarray(W2, np.float32).astype(BF16),
        "w3s": np.asarray(w3_self, np.float32).astype(BF16),
        "wq": (headcat(Wq) * scale).astype(BF16),
        "we": np.ascontiguousarray(
            np.transpose(np.asarray(We, np.float32), (1, 0, 2))
            .reshape(OTH_IN, A * H_OTH)).astype(BF16),
        "wk": headcat(Wk).astype(BF16),
        "wv": headcat(Wv).astype(BF16),
        "w3o": np.asarray(w3_others, np.float32).astype(BF16),
        "wout": np.asarray(Wout, np.float32).astype(BF16),
        "sind": sind.reshape(H_OTH, A * LROWS),
        "wl": wl.reshape(28, A * H_OTH),
    }
    vals_f32 = {
        "b1": np.asarray(b1, np.float32).reshape(H_SELF, 1),
        "b2": np.asarray(b2, np.float32).reshape(H_SELF, 1),
        "be": np.ascontiguousarray(np.asarray(be, np.float32).T),
        "bv": np.asarray(bv, np.float32).reshape(HEADS * AD, 1),
        "bout": np.asarray(bout, np.float32).reshape(1, 1),
    }

    def pack(spec, values, dtype):
        off, width = _pack_layout(spec)
        arr = np.zeros((128, width), dtype=dtype)
        for name, rows, cols in spec:
            v = values[name]
            assert v.shape == (rows, cols), (name, v.shape, rows, cols)
            arr[:rows, off[name]:off[name] + cols] = v
        return arr

    cb = pack(_CONSTS_BF16, vals_bf16, BF16)
    cf = pack(_CONSTS_F32, vals_f32, np.float32)

    in_maps = []
    for c in range(NCORES):
        sl = slice(c * BC, (c + 1) * BC)
        m = {"cb": cb, "cf": cf,
             "ot": np.ascontiguousarray(ot_full[:, :, sl])}
        in_maps.append(m)
    return in_maps


def get_nc(reps=1):
    key = ("nc", reps)
    if key not in _CACHE:
        _CACHE[key] = _build_nc(reps)
    return _CACHE[key]


def kernel(**inputs) -> np.ndarray:
    from concourse.bass_utils import run_bass_kernel_spmd

    nc = get_nc()
    in_maps = _prep_inputs(**inputs)
    res = run_bass_kernel_spmd(nc, in_maps, core_ids=list(range(NCORES)))
    out = np.concatenate(
        [np.asarray(res.results[c]["out"], np.float32).reshape(BC, 1)
         for c in range(NCORES)], axis=0)
    return out
